# revision 10
# baseline (speedup 1.0000x reference)
"""Bass/Trainium2 kernel for a 7-step LSTM decoder (nn_Decoder_19705309954222).

    zx_t = x_t @ Wk + b ; z_t = zx_t + h_{t-1} @ Wr
    i,f,g,o = sig/tanh gates(z_t) ; c_t = f*c + i*g ; h_t = o*tanh(c_t)
    output: h_t for t=0..6, shape [B, T, U].

Sharding: data-parallel over 8 NeuronCores (batch 32768 -> 4096/core), weights
replicated. On-device layout is fully transposed: hidden state h^T is [U=256,
batch], kept as ONE SBUF tile [128, 2*BC] with the two 128-row halves of U side
by side in the free dim; gate pre-activations are 4 PSUM tiles [128, 2*BC] (one
per gate). The recurrent matmul keeps Wr/Wk stationary and streams the batch as
the moving operand; the input projection AND the bias (as a constant-1 row on x,
K=36->37) are fused into the same PSUM accumulation group. The host
pre-transposes x/h0/c0 and re-transposes the output so the device does zero
transposes and all DMAs are wide contiguous rows.
"""

import os
import numpy as np
import ml_dtypes

import concourse.bacc as bacc
import concourse.mybir as mybir
import concourse.tile as tile
from concourse.bass_utils import run_bass_kernel_spmd

B, T, F, U = 32768, 7, 36, 256
FK = F + 1  # x augmented with a constant-1 row; Wk augmented with bias row
G = 4 * U  # 1024
N_CORES = 8
BL = B // N_CORES  # 4096 batch rows per core
M_TILES = G // 128  # 8; gate gi covers m = 2*gi (+0/+1)

# dtype / size config (dev override via env; defaults = shipped config)
MM_DT_NAME = os.environ.get("LSTM_MM_DT", "f16")  # matmul operands: f32|f32r|bf16
GATE_DT_NAME = os.environ.get("LSTM_GATE_DT", "f16")  # i,f,g,o and tanh(c) tiles
CELL_DT_NAME = os.environ.get("LSTM_CELL_DT", "f16")  # c tiles
OUT_DT_NAME = os.environ.get("LSTM_OUT_DT", "f16")  # h tiles / DRAM output
BC = int(os.environ.get("LSTM_BC", "512"))  # batch columns per chunk
PROBE2X = os.environ.get("LSTM_2X", "")  # "", "act", "pe", "dve": double that engine's work
ILV = int(os.environ.get("LSTM_ILV", "2"))  # chunks processed round-robin
SAMEW = os.environ.get("LSTM_SAMEW", "") == "1"  # timing-only probe: reuse one Wr tile
NOWK = os.environ.get("LSTM_NOWK", "") == "1"  # timing-only probe: skip Wk matmuls
NOH = os.environ.get("LSTM_NOH", "") == "1"  # timing-only probe: break h recurrence
NOOUT = os.environ.get("LSTM_NOOUT", "") == "1"  # timing-only probe: skip out DMA
NOX = os.environ.get("LSTM_NOX", "") == "1"  # timing-only probe: skip x DMA

_DT = {
    "f32": mybir.dt.float32,
    "f32r": mybir.dt.float32r,
    "bf16": mybir.dt.bfloat16,
    "f16": mybir.dt.float16,
}
_NP = {"f32": np.float32, "f32r": np.float32, "bf16": ml_dtypes.bfloat16,
       "f16": np.float16}

f32 = mybir.dt.float32


def _build_program(reps=1, mm=None, gate=None, cell=None, out=None, bc=None,
                   probe2x=None, ilv=None, fuse_if=False):
    mm = mm or MM_DT_NAME
    gate = gate or GATE_DT_NAME
    cell = cell or CELL_DT_NAME
    out = out or OUT_DT_NAME
    bc = bc or BC
    probe2x = PROBE2X if probe2x is None else probe2x
    ilv = ILV if ilv is None else ilv
    gate_dt = _DT[gate]
    cell_dt = _DT[cell]
    out_dt = _DT[out]
    # dtype stored in DRAM/SBUF for matmul inputs (f32r is fp32 bits; engines
    # round on write so the BIR verifier sees f32r producers)
    io_dt = _DT[mm]
    h_mm_dt = io_dt  # h feedback into next step's matmul
    BCL = bc

    nc = bacc.Bacc("TRN2", target_bir_lowering=False, debug=False)

    xT_d = nc.dram_tensor("xT", [T, 2 * FK, BL], io_dt, kind="ExternalInput")
    h0T_d = nc.dram_tensor("h0T", [U, BL], io_dt, kind="ExternalInput")
    c0T_d = nc.dram_tensor("c0T", [U, BL], cell_dt, kind="ExternalInput")
    # wr: col block (k*8+m) = Wr[k*128:(k+1)*128, m*128:(m+1)*128]
    wr_d = nc.dram_tensor("wr", [128, 2 * M_TILES * 128], io_dt, kind="ExternalInput")
    wk_d = nc.dram_tensor("wk", [128, G], io_dt, kind="ExternalInput")
    out_d = nc.dram_tensor("hsT", [T, U, BL], out_dt, kind="ExternalOutput")

    def mm_ap(ap):
        return ap

    n_chunks = BL // BCL
    Sig = mybir.ActivationFunctionType.Sigmoid
    Tanh = mybir.ActivationFunctionType.Tanh
    MUL = mybir.AluOpType.mult
    ADD = mybir.AluOpType.add
    GATE_FUNCS = [Sig, Sig, Tanh, Sig]  # i, f, g, o

    with tile.TileContext(nc) as tc:
        with (
            tc.tile_pool(name="w", bufs=1) as wp,
            tc.tile_pool(name="x", bufs=1 if mm in ("f32", "f32r") else 2) as xp,
            tc.tile_pool(name="state", bufs=2) as sp,
            tc.tile_pool(name="gates", bufs=2) as gp,
            tc.tile_pool(name="ew", bufs=2) as ep,
            tc.tile_pool(name="z", bufs=1, space="PSUM") as zp,
        ):
            wr_t = wp.tile([128, 2 * M_TILES * 128], io_dt, tag="wr")
            nc.sync.dma_start(wr_t[:], wr_d.ap())
            wk_t = wp.tile([128, G], io_dt, tag="wk")
            nc.sync.dma_start(wk_t[:], wk_d.ap())

            def wr_ap(k, m):
                if SAMEW:
                    return mm_ap(wr_t[:, 0:128])
                j = (k * M_TILES + m) * 128
                return mm_ap(wr_t[:, j:j + 128])

            def wk_ap(m):
                # even m lives at partitions 0:FK, odd m at 64:64+FK
                base = 0 if m % 2 == 0 else 64
                return mm_ap(wk_t[base:base + FK, m * 128:(m + 1) * 128])

            def emit_chunk_load(ci):
                b0 = ci * BCL
                par = ci % ilv
                x_t = xp.tile([64 + FK, T * BCL], io_dt, tag=f"x{par}")
                for t in range(T if not NOX else 0):
                    nc.sync.dma_start(
                        x_t[0:FK, t * BCL:(t + 1) * BCL],
                        xT_d.ap()[t, 0:FK, b0:b0 + BCL],
                    )
                    nc.sync.dma_start(
                        x_t[64:64 + FK, t * BCL:(t + 1) * BCL],
                        xT_d.ap()[t, FK:2 * FK, b0:b0 + BCL],
                    )
                h = sp.tile([128, 2 * BCL], h_mm_dt, tag=f"h{par}")
                c = sp.tile([128, 2 * BCL], cell_dt, tag=f"c{par}")
                for p in range(2):
                    nc.sync.dma_start(
                        h[:, p * BCL:(p + 1) * BCL],
                        h0T_d.ap()[p * 128:(p + 1) * 128, b0:b0 + BCL],
                    )
                    nc.sync.dma_start(
                        c[:, p * BCL:(p + 1) * BCL],
                        c0T_d.ap()[p * 128:(p + 1) * 128, b0:b0 + BCL],
                    )
                return {"x": x_t, "h": h, "c": c, "b0": b0, "ci": ci}

            def emit_step(st, t):
                ci, b0 = st["ci"], st["b0"]
                par = ci % ilv
                x_t, h_prev, c_prev = st["x"], st["h"], st["c"]
                xt_lo = mm_ap(x_t[0:FK, t * BCL:(t + 1) * BCL])
                xt_hi = mm_ap(x_t[64:64 + FK, t * BCL:(t + 1) * BCL])

                # gate pre-activations: 4 PSUM tiles [128, 2*BCL]; halves are the
                # two 128-row U-blocks (m = 2*gi + p), each a 3-matmul accum group
                z = [None] * 4
                zif = None
                if fuse_if:
                    zif = zp.tile([128, 4 * BCL], f32, tag="zif")
                # order i,g,f,o: the DVE chain consumes i*g first, f next,
                # o last, so start its inputs as early as possible
                for gi in ([0, 1, 2, 3] if fuse_if else [0, 2, 1, 3]):
                    if fuse_if and gi < 2:
                        zg = zif[:, gi * 2 * BCL:(gi + 1) * 2 * BCL]
                    else:
                        zg = zp.tile([128, 2 * BCL], f32, tag=f"z{gi}")
                    # the two K=37 projections run concurrently in PE row
                    # groups (0,0)/(64,0); they have no h dependency, so they
                    # lead the accumulation group
                    if not NOWK:
                        for p in range(2):
                            m = 2 * gi + p
                            zs = zg[:, p * BCL:(p + 1) * BCL]
                            nc.tensor.matmul(
                                zs, wk_ap(m), xt_lo if p == 0 else xt_hi,
                                start=True, stop=False,
                                tile_position=(0 if p == 0 else 64, 0),
                            )
                    for p in range(2):
                        m = 2 * gi + p
                        zs = zg[:, p * BCL:(p + 1) * BCL]
                        for _du in range(2 if probe2x == "pe" else 1):
                            nc.tensor.matmul(
                                zs, wr_ap(0, m), mm_ap(h_prev[:, 0:BCL]),
                                start=NOWK, stop=False,
                            )
                            nc.tensor.matmul(
                                zs, wr_ap(1, m), mm_ap(h_prev[:, BCL:2 * BCL]),
                                start=False, stop=True,
                            )
                    z[gi] = zg

                if fuse_if:
                    if_t = gp.tile([128, 4 * BCL], gate_dt, tag=f"gif_{par}")
                    nc.scalar.activation(if_t[:], zif[:], Sig)
                    g_t = gp.tile([128, 2 * BCL], gate_dt, tag=f"g2_{par}")
                    nc.scalar.activation(g_t[:], z[2][:], Tanh)
                    o_t = gp.tile([128, 2 * BCL], gate_dt, tag=f"g3_{par}")
                    nc.scalar.activation(o_t[:], z[3][:], Sig)
                    i_t = if_t[:, 0:2 * BCL]
                    f_t = if_t[:, 2 * BCL:4 * BCL]
                else:
                    gt = [None] * 4
                    for gi in [0, 2, 1, 3]:
                        g_t = gp.tile([128, 2 * BCL], gate_dt, tag=f"g{gi}_{par}")
                        for _du in range(2 if probe2x == "act" else 1):
                            nc.scalar.activation(g_t[:], z[gi][:], GATE_FUNCS[gi])
                        gt[gi] = g_t
                    i_t, f_t, g_t, o_t = gt

                dve_n = 2 if probe2x == "dve" else 1
                ig = ep.tile([128, 2 * BCL], gate_dt, tag=f"ig{par}")
                for _du in range(dve_n):
                    nc.vector.tensor_tensor(ig[:], i_t if fuse_if else i_t[:], g_t[:], MUL)
                cn = sp.tile([128, 2 * BCL], cell_dt, tag=f"c{par}")
                for _du in range(dve_n):
                    nc.vector.tensor_tensor(cn[:], f_t if fuse_if else f_t[:], c_prev[:], MUL)
                nc.vector.tensor_tensor(cn[:], cn[:], ig[:], ADD)
                if dve_n == 2:
                    nc.vector.tensor_tensor(cn[:], cn[:], ig[:], MUL)
                tc_t = ep.tile([128, 2 * BCL], gate_dt, tag=f"tc{par}")
                nc.scalar.activation(tc_t[:], cn[:], Tanh)
                hn = sp.tile([128, 2 * BCL], out_dt, tag=f"ho{par}")
                for _du in range(dve_n):
                    nc.vector.tensor_tensor(hn[:], o_t[:], tc_t[:], MUL)
                for p in range(2 if not NOOUT else 0):
                    nc.sync.dma_start(
                        out_d.ap()[t, p * 128:(p + 1) * 128, b0:b0 + BCL],
                        hn[:, p * BCL:(p + 1) * BCL],
                    )
                if out_dt == h_mm_dt:
                    h_mm = hn
                else:
                    h_mm = sp.tile([128, 2 * BCL], h_mm_dt, tag=f"h{par}")
                    nc.vector.tensor_copy(h_mm[:], hn[:])
                if not NOH:
                    st["h"] = h_mm
                st["c"] = cn

            def emit_body():
                for grp in range(0, n_chunks, ilv):
                    sts = [emit_chunk_load(ci)
                           for ci in range(grp, min(grp + ilv, n_chunks))]
                    for t in range(T):
                        for st in sts:
                            emit_step(st, t)

            if reps == 1:
                emit_body()
            else:
                # hardware loop: program size stays constant as reps grows, so
                # wall-clock differencing vs reps=1 isolates device exec time
                # (per-call program upload cost cancels)
                with tc.For_i(0, reps):
                    emit_body()

    nc.compile()
    return nc


_PROGRAM = None


def _get_program():
    global _PROGRAM
    if _PROGRAM is None:
        _PROGRAM = _build_program(int(os.environ.get("LSTM_REPS", "1")))
    return _PROGRAM


def _prep_inputs(x, h0, c0, Wk, Wr, b):
    io_np = _NP[MM_DT_NAME]
    wr_host = np.ascontiguousarray(
        Wr.reshape(2, 128, M_TILES, 128).transpose(1, 0, 2, 3)
        .reshape(128, 2 * M_TILES * 128)
    ).astype(io_np)
    wk_aug = np.concatenate([Wk, b[None, :]], axis=0)  # [37, 1024]
    wk_host = np.zeros((128, G), dtype=np.float32)
    for m in range(M_TILES):
        base = 0 if m % 2 == 0 else 64
        wk_host[base:base + FK, m * 128:(m + 1) * 128] = wk_aug[:, m * 128:(m + 1) * 128]
    wk_host = wk_host.astype(io_np)
    in_maps = []
    for i in range(N_CORES):
        s = slice(i * BL, (i + 1) * BL)
        xT = np.empty((T, 2 * FK, BL), dtype=io_np)
        xT[:, :F, :] = x[s].transpose(1, 2, 0).astype(io_np)
        xT[:, F, :] = np.float32(1.0)
        xT[:, FK:, :] = xT[:, :FK, :]
        h0T = np.ascontiguousarray(h0[s].T).astype(io_np)
        c0T = np.ascontiguousarray(c0[s].T).astype(_NP[CELL_DT_NAME])
        in_maps.append(
            {"xT": xT, "h0T": h0T, "c0T": c0T, "wr": wr_host, "wk": wk_host}
        )
    return in_maps


def _gather_output(results):
    outs = []
    for i in range(N_CORES):
        hsT = np.asarray(results[i]["hsT"]).astype(np.float32)  # [T, U, BL]
        outs.append(hsT.transpose(2, 0, 1))  # [BL, T, U]
    return np.ascontiguousarray(np.concatenate(outs, axis=0))


def kernel(x, h0, c0, Wk, Wr, b, _trace=False):
    x = np.asarray(x, dtype=np.float32)
    h0 = np.asarray(h0, dtype=np.float32)
    c0 = np.asarray(c0, dtype=np.float32)
    Wk = np.asarray(Wk, dtype=np.float32)
    Wr = np.asarray(Wr, dtype=np.float32)
    b = np.asarray(b, dtype=np.float32)

    nc = _get_program()
    in_maps = _prep_inputs(x, h0, c0, Wk, Wr, b)
    res = run_bass_kernel_spmd(
        nc, in_maps, core_ids=list(range(N_CORES)), trace=_trace
    )
    out = _gather_output(res.results)
    if _trace:
        return out, res
    return out

